# revision 7
# baseline (speedup 1.0000x reference)
"""Trainium2 Bass kernel for MixedMoEProjectionLayer.

Strategy: data-parallel over the 8192-token batch across 8 NeuronCores
(1024 tokens per core), expert params replicated. Per core everything is
computed in a feature-major layout (features on partitions, tokens on the
free axis) so no activation transposes are needed anywhere:

    out.T[f_out, t] = W[f_in, f_out].T @ h.T[f_in, t]

- Matmuls run in fp16 (weights host-cast, activations stored fp16) with
  fp32 PSUM accumulation. The gate runs in full fp32 (top-2 selection is
  discrete; precision matters there).
- LayerNorm is over features (= partitions), so stats are computed with
  ones-vector matmuls accumulated in PSUM during PSUM eviction, and the
  normalization (h*a + c per token column) is applied lazily when the next
  layer streams its input tiles. GELU's 0.5 factor is folded into the
  following LN by scaling eps by 4 (exact algebra).
- The per-token top-2 gate weights fold into each expert's final LN scale;
  per-expert contributions are written to a [4*1024, 1024] output and
  summed on host.
"""

import math
import os
from contextlib import ExitStack

import numpy as np

N_CORES = 8
LN_EPS = 1e-5

FULL_CFG = dict(
    d_in=1024,
    T=1024,  # tokens per core
    experts=[
        dict(hids=[2048], act="gelu"),
        dict(hids=[4096, 4096], act="silu"),
        dict(hids=[6144, 6144, 6144], act="relu"),
        dict(hids=[2048], act="lrelu"),
    ],
)

M_BLK = 3  # m-tiles per psum block (6 psum banks for main matmuls)


def _blocks(mt, blk=M_BLK):
    out, m0 = [], 0
    while m0 < mt:
        bw = min(blk, mt - m0)
        out.append((m0, bw))
        m0 += bw
    return out


def build_moe(cfg):
    """Build the Bass program for one core. Returns (nc, meta)."""
    import concourse.bass as bass
    import concourse.mybir as mybir
    import concourse.tile as tile
    from concourse import bacc
    from concourse.masks import make_identity

    dt = mybir.dt
    f32, f16 = dt.float32, dt.float16
    AF = mybir.ActivationFunctionType
    OP = mybir.AluOpType
    AX = mybir.AxisListType

    d_in = cfg["d_in"]
    T = cfg["T"]
    experts = cfg["experts"]
    P = 128
    NB = min(512, T)               # n-slice width (psum bank limit for fp32)
    NSL = (T + NB - 1) // NB       # n slices (<= 2 so 2*NSL <= 4 stat chains)
    assert T % NB == 0 and NSL * 2 <= 4
    Kt_in = d_in // P
    max_kt = max(max([d_in] + list(ex["hids"])) // P for ex in experts)

    nc = bacc.Bacc("TRN2", target_bir_lowering=False, debug=False)

    # ---------------- DRAM I/O ----------------
    xt16 = nc.dram_tensor("xt16", [d_in, T], f16, kind="ExternalInput").ap()
    xt32 = nc.dram_tensor("xt32", [d_in, T], f32, kind="ExternalInput").ap()
    gw = nc.dram_tensor("gate_w", [d_in, 4], f32, kind="ExternalInput").ap()
    gb = nc.dram_tensor("gate_b", [4], f32, kind="ExternalInput").ap()

    wins, bins, berfs, hdrams = {}, {}, {}, {}
    for e, ex in enumerate(experts):
        dims = [d_in] + list(ex["hids"]) + [d_in]
        for l in range(len(dims) - 1):
            din, dout = dims[l], dims[l + 1]
            wins[(e, l)] = nc.dram_tensor(
                f"w_e{e}_l{l}", [din, dout], f16, kind="ExternalInput"
            ).ap()
            bins[(e, l)] = nc.dram_tensor(
                f"b_e{e}_l{l}", [P, dout // P], f32, kind="ExternalInput"
            ).ap()
            if ex["act"] == "gelu" and l < len(dims) - 2:
                berfs[(e, l)] = nc.dram_tensor(
                    f"berf_e{e}_l{l}", [P, dout // P], f32, kind="ExternalInput"
                ).ap()
            if l < len(dims) - 2:  # hidden-layer output scratch in DRAM
                hdrams[(e, l)] = nc.dram_tensor(f"h_e{e}_l{l}", [dout, T], f16).ap()
    outT4 = nc.dram_tensor("outT4", [4 * d_in, T], f32, kind="ExternalOutput").ap()

    with ExitStack() as ctx:
        tc = ctx.enter_context(tile.TileContext(nc))

        h_pool = ctx.enter_context(tc.tile_pool(name="h", bufs=max_kt))
        f32_pool = ctx.enter_context(tc.tile_pool(name="f32p", bufs=4))
        zf_pool = ctx.enter_context(tc.tile_pool(name="zf", bufs=d_in // P))
        w_pool = ctx.enter_context(tc.tile_pool(name="w", bufs=5))
        hout_pool = ctx.enter_context(tc.tile_pool(name="hout", bufs=4))
        sq_pool = ctx.enter_context(tc.tile_pool(name="sq", bufs=2))
        bias_pool = ctx.enter_context(tc.tile_pool(name="bias", bufs=4))
        bc_pool = ctx.enter_context(tc.tile_pool(name="bc", bufs=3))
        rows_pool = ctx.enter_context(tc.tile_pool(name="rows", bufs=3))
        srow_pool = ctx.enter_context(tc.tile_pool(name="srow", bufs=1))
        gate_pool = ctx.enter_context(tc.tile_pool(name="gate", bufs=2))
        const_pool = ctx.enter_context(tc.tile_pool(name="const", bufs=1))
        main_ps = ctx.enter_context(
            tc.tile_pool(name="mainps", bufs=2 * M_BLK, space="PSUM")
        )
        aux_ps = ctx.enter_context(tc.tile_pool(name="auxps", bufs=2, space="PSUM"))

        # ---------------- constants ----------------
        ident = const_pool.tile([P, P], f32, tag="ident")
        make_identity(nc, ident)
        ones_row = const_pool.tile([1, P], f32, tag="onesrow")
        nc.vector.memset(ones_row, 1.0)
        ones4 = const_pool.tile([P, 4, 4], f16, tag="ones4")
        nc.vector.memset(ones4, 0.0)
        for ci in range(4):
            nc.vector.memset(ones4[:, ci, ci : ci + 1], 1.0)
        gb_sb = const_pool.tile([4, 1], f32, tag="gbsb")
        nc.sync.dma_start(out=gb_sb, in_=gb)
        w_fm = const_pool.tile([4, T], f32, tag="wfm")
        lg_fm = const_pool.tile([4, T], f32, tag="lgfm")
        eps_t = const_pool.tile([1, 1], f32, tag="eps1")
        nc.vector.memset(eps_t, LN_EPS)
        eps4_t = const_pool.tile([1, 1], f32, tag="eps4")
        nc.vector.memset(eps4_t, 4.0 * LN_EPS)

        # ---------------- gate: logits (feature-major), softmax+top2 -------
        ps_g = [
            aux_ps.tile([P, NB], f32, tag="aux", name=f"psg{n}") for n in range(NSL)
        ]
        for k in range(Kt_in):
            gwk = gate_pool.tile([P, 4], f32, tag="gw")
            nc.sync.dma_start(out=gwk, in_=gw[k * P : (k + 1) * P, :])
            xk = f32_pool.tile([P, T], f32, tag="f32")
            nc.sync.dma_start(out=xk, in_=xt32[k * P : (k + 1) * P, :])
            for n in range(NSL):
                nc.tensor.matmul(
                    ps_g[n][0:4, 0:NB],
                    gwk,
                    xk[:, n * NB : (n + 1) * NB],
                    start=(k == 0),
                    stop=(k == Kt_in - 1),
                )
        for n in range(NSL):
            nc.scalar.activation(
                lg_fm[:, n * NB : (n + 1) * NB],
                ps_g[n][0:4, 0:NB],
                AF.Identity,
                bias=gb_sb,
            )
        for j in range(T // P):
            tp1 = aux_ps.tile([P, NB], f32, tag="aux")
            nc.tensor.transpose(
                tp1[0:P, 0:4], lg_fm[:, j * P : (j + 1) * P], ident[0:4, 0:4]
            )
            lg = gate_pool.tile([P, 4], f32, tag="lg")
            nc.scalar.copy(lg, tp1[0:P, 0:4])
            nm1 = gate_pool.tile([P, 1], f32, tag="nm1")
            nc.vector.reduce_max(nm1, lg, axis=AX.X, negate=True)
            et = gate_pool.tile([P, 4], f32, tag="et")
            nc.scalar.activation(et, lg, AF.Exp, bias=nm1, scale=1.0)
            ssum = gate_pool.tile([P, 1], f32, tag="ssum")
            nc.vector.reduce_sum(ssum, et, axis=AX.X)
            rinv = gate_pool.tile([P, 1], f32, tag="rinv")
            nc.vector.reciprocal(rinv, ssum)
            p = gate_pool.tile([P, 4], f32, tag="p")
            nc.vector.tensor_scalar_mul(p, et, rinv)
            mx = gate_pool.tile([P, 1], f32, tag="mx")
            nc.vector.reduce_max(mx, p, axis=AX.X)
            ismax = gate_pool.tile([P, 4], f32, tag="ismax")
            nc.vector.tensor_scalar(ismax, p, mx, None, OP.is_ge)
            pm = gate_pool.tile([P, 4], f32, tag="pm")
            nc.vector.scalar_tensor_tensor(pm, ismax, -2.0, p, OP.mult, OP.add)
            m2 = gate_pool.tile([P, 1], f32, tag="m2")
            nc.vector.reduce_max(m2, pm, axis=AX.X)
            mask = gate_pool.tile([P, 4], f32, tag="mask")
            nc.vector.tensor_scalar(mask, p, m2, None, OP.is_ge)
            wgt = gate_pool.tile([P, 4], f32, tag="wgt")
            nc.vector.tensor_tensor(wgt, p, mask, OP.mult)
            tp2 = aux_ps.tile([P, NB], f32, tag="aux")
            nc.tensor.transpose(tp2[0:4, 0:P], wgt, ident)
            nc.scalar.copy(w_fm[:, j * P : (j + 1) * P], tp2[0:4, 0:P])

        # ---------------- expert layers ----------------
        def bcast_row(row, name):
            """[1, T] f32 row -> [128, T] f16 broadcast tile."""
            bt = bc_pool.tile([P, T], f16, tag="bc", name=name)
            for n in range(NSL):
                ps = aux_ps.tile([P, NB], f32, tag="aux")
                nc.tensor.matmul(
                    ps[:, 0:NB],
                    ones_row,
                    row[:, n * NB : (n + 1) * NB],
                    start=True,
                    stop=True,
                )
                nc.scalar.copy(bt[:, n * NB : (n + 1) * NB], ps[:, 0:NB])
            return bt

        for e, ex in enumerate(experts):
            dims = [d_in] + list(ex["hids"]) + [d_in]
            nlayers = len(dims) - 1
            act = ex["act"]
            a_b = c_b = None
            src_tiles = None

            for l in range(nlayers):
                din, dout = dims[l], dims[l + 1]
                Kt, Mt = din // P, dout // P
                is_final = l == nlayers - 1
                is_gelu_layer = act == "gelu" and not is_final
                eps_eff = eps4_t if is_gelu_layer else eps_t

                # source tiles
                new_tiles = []
                for k in range(Kt):
                    ht = h_pool.tile([P, T], f16, tag="hcache", name=f"h{e}_{l}_{k}")
                    if l == 0:
                        nc.sync.dma_start(out=ht, in_=xt16[k * P : (k + 1) * P, :])
                    else:
                        nc.sync.dma_start(
                            out=ht, in_=hdrams[(e, l - 1)][k * P : (k + 1) * P, :]
                        )
                        nc.vector.tensor_tensor(ht, ht, a_b, OP.mult)
                        nc.vector.tensor_tensor(ht, ht, c_b, OP.add)
                    new_tiles.append(ht)
                src_tiles = new_tiles

                # biases
                b_sb = bias_pool.tile([P, Mt], f32, tag="bias")
                nc.sync.dma_start(out=b_sb, in_=bins[(e, l)])
                berf_sb = None
                if is_gelu_layer:
                    berf_sb = bias_pool.tile([P, Mt], f32, tag="bias")
                    nc.sync.dma_start(out=berf_sb, in_=berfs[(e, l)])

                W = wins[(e, l)]
                stats_ps = aux_ps.tile([4, NB], f32, tag="aux")
                n_stat_mms = Mt * 2 * NSL
                stat_i = [0]
                zf_tiles = []

                def stat_mm(ci, rhs):
                    nc.tensor.matmul(
                        stats_ps,
                        ones4[:, ci, :],
                        rhs,
                        start=(stat_i[0] == 0),
                        stop=(stat_i[0] == n_stat_mms - 1),
                    )
                    stat_i[0] += 1

                def evict(m, psums):
                    bcol = b_sb[:, m : m + 1]
                    if is_final:
                        dst_t = zf_pool.tile([P, T], f16, tag="zf", name=f"zf{e}_{m}")
                    else:
                        dst_t = hout_pool.tile([P, T], f16, tag="hout")
                    sq_t = sq_pool.tile([P, T], f16, tag="sq")
                    if act in ("silu", "gelu") and not is_final:
                        zt = f32_pool.tile([P, T], f32, tag="f32")
                        st = f32_pool.tile([P, T], f32, tag="f32")
                    for n in range(NSL):
                        sl = slice(n * NB, (n + 1) * NB)
                        dst = dst_t[:, sl]
                        if is_final:
                            nc.scalar.activation(dst, psums[n], AF.Identity, bias=bcol)
                        elif act == "relu":
                            nc.scalar.activation(dst, psums[n], AF.Relu, bias=bcol)
                        elif act == "lrelu":
                            nc.scalar.activation(
                                dst, psums[n], AF.Lrelu, bias=bcol, alpha=0.01
                            )
                        elif act == "silu":
                            nc.scalar.activation(
                                st[:, sl], psums[n], AF.Sigmoid, bias=bcol
                            )
                            nc.vector.tensor_scalar_add(zt[:, sl], psums[n], bcol)
                            nc.vector.tensor_tensor(dst, zt[:, sl], st[:, sl], OP.mult)
                        elif act == "gelu":
                            nc.scalar.activation(
                                st[:, sl],
                                psums[n],
                                AF.Erf,
                                bias=berf_sb[:, m : m + 1],
                                scale=0.7071067811865476,
                            )
                            nc.vector.tensor_scalar_add(zt[:, sl], psums[n], bcol)
                            nc.vector.tensor_tensor(
                                st[:, sl], zt[:, sl], st[:, sl], OP.mult
                            )
                            # z + z*erf = 2*gelu (0.5 folded into next LN eps)
                            nc.vector.tensor_tensor(dst, zt[:, sl], st[:, sl], OP.add)
                        nc.vector.tensor_tensor(sq_t[:, sl], dst, dst, OP.mult)
                        stat_mm(n, dst)
                        stat_mm(NSL + n, sq_t[:, sl])
                    if is_final:
                        zf_tiles.append(dst_t)
                    else:
                        nc.sync.dma_start(
                            out=hdrams[(e, l)][m * P : (m + 1) * P, :], in_=dst_t
                        )

                for m0, bw in _blocks(Mt):
                    psums = [
                        main_ps.tile([P, NB], f32, tag="mm", name=f"mmps{i}")
                        for i in range(bw * NSL)
                    ]
                    for k in range(Kt):
                        wt = w_pool.tile([P, M_BLK * P], f16, tag="w")
                        nc.sync.dma_start(
                            out=wt[:, 0 : bw * P],
                            in_=W[k * P : (k + 1) * P, m0 * P : (m0 + bw) * P],
                        )
                        for j in range(bw):
                            for n in range(NSL):
                                nc.tensor.matmul(
                                    psums[j * NSL + n],
                                    wt[:, j * P : (j + 1) * P],
                                    src_tiles[k][:, n * NB : (n + 1) * NB],
                                    start=(k == 0),
                                    stop=(k == Kt - 1),
                                )
                    for j in range(bw):
                        evict(m0 + j, psums[j * NSL : (j + 1) * NSL])

                # ---- layer end: stats -> a/c rows (in place in srow) ----
                st_sb = gate_pool.tile([4, NB], f32, tag="stsb")
                nc.scalar.copy(st_sb, stats_ps)
                srow = srow_pool.tile([1, 4 * NB], f32, tag="rows4")
                nc.sync.dma_start(out=srow, in_=st_sb)
                # srow layout: [h_n0 | h_n1 | sq_n0 | sq_n1] each NB wide
                sh = srow[:, 0:T]
                ss = srow[:, 2 * NB : 2 * NB + T] if NSL == 2 else srow[:, NB : NB + T]
                # mu in place of sh
                nc.vector.tensor_scalar_mul(sh, sh, 1.0 / dout)
                t1 = rows_pool.tile([1, T], f32, tag="rows")
                nc.vector.tensor_tensor(t1, sh, sh, OP.mult)  # mu^2
                # var = ss/dout - mu^2, in place of ss
                nc.vector.scalar_tensor_tensor(
                    ss, ss, 1.0 / dout, t1, OP.mult, OP.subtract
                )
                nc.scalar.activation(ss, ss, AF.Sqrt, bias=eps_eff[0:1, :])
                nc.vector.reciprocal(ss, ss)  # rs in place
                # c = -mu * rs
                nc.vector.scalar_tensor_tensor(t1, sh, -1.0, ss, OP.mult, OP.mult)

                if not is_final:
                    a_b = bcast_row(ss, "a_b")
                    c_b = bcast_row(t1, "c_b")
                else:
                    wrow = rows_pool.tile([1, T], f32, tag="rows")
                    nc.sync.dma_start(out=wrow, in_=w_fm[e : e + 1, :])
                    arow = rows_pool.tile([1, T], f32, tag="rows")
                    nc.vector.tensor_tensor(arow, ss, wrow, OP.mult)
                    nc.vector.tensor_tensor(t1, t1, wrow, OP.mult)
                    a_bf = bcast_row(arow, "a_b")
                    c_bf = bcast_row(t1, "c_b")
                    for m in range(Mt):
                        o1 = f32_pool.tile([P, T], f32, tag="f32")
                        nc.vector.tensor_tensor(o1, zf_tiles[m], a_bf, OP.mult)
                        nc.vector.tensor_tensor(o1, o1, c_bf, OP.add)
                        nc.sync.dma_start(
                            out=outT4[e * d_in + m * P : e * d_in + (m + 1) * P, :],
                            in_=o1,
                        )

    nc.compile()
    return nc, dict(cfg=cfg)


# ---------------------------------------------------------------------------
# host side
# ---------------------------------------------------------------------------
_BUILT = {}


def _get_built():
    if "nc" not in _BUILT:
        os.environ.setdefault("JAX_PLATFORMS", "axon")
        nc, meta = build_moe(FULL_CFG)
        _BUILT["nc"] = nc
        _BUILT["meta"] = meta
    return _BUILT["nc"], _BUILT["meta"]


def _prep_in_maps(x, gate_w, gate_b, params, cfg, n_cores=N_CORES):
    x = np.asarray(x, dtype=np.float32)
    gate_w = np.asarray(gate_w, dtype=np.float32)
    gate_b = np.asarray(gate_b, dtype=np.float32)
    B, d_in = x.shape
    T = cfg["T"]
    P = 128
    assert B == n_cores * T

    shared = {
        "gate_w": np.ascontiguousarray(gate_w),
        "gate_b": np.ascontiguousarray(gate_b),
    }
    for e, (ex, p) in enumerate(zip(cfg["experts"], params)):
        dims = [d_in] + list(ex["hids"]) + [d_in]
        mats = [
            (layer["w"], layer["b"], layer["g"], layer["beta"])
            for layer in p["hidden"]
        ]
        mats.append((p["w_out"], p["b_out"], p["g_out"], p["beta_out"]))
        for l, (w, b, g, beta) in enumerate(mats):
            w = np.asarray(w, dtype=np.float32)
            b = np.asarray(b, dtype=np.float32)
            g = np.asarray(g, dtype=np.float32)
            beta = np.asarray(beta, dtype=np.float32)
            if not (np.all(g == 1.0) and np.all(beta == 0.0)):
                raise ValueError("kernel assumes LN gamma=1, beta=0")
            dout = dims[l + 1]
            Mt = dout // P
            shared[f"w_e{e}_l{l}"] = np.ascontiguousarray(w.astype(np.float16))
            shared[f"b_e{e}_l{l}"] = np.ascontiguousarray(
                b.reshape(Mt, P).T.astype(np.float32)
            )
            if ex["act"] == "gelu" and l < len(dims) - 2:
                shared[f"berf_e{e}_l{l}"] = np.ascontiguousarray(
                    (b / math.sqrt(2.0)).reshape(Mt, P).T.astype(np.float32)
                )

    in_maps = []
    for c in range(n_cores):
        xc = x[c * T : (c + 1) * T]
        xT32 = np.ascontiguousarray(xc.T)
        m = dict(shared)
        m["xt32"] = xT32
        m["xt16"] = xT32.astype(np.float16)
        in_maps.append(m)
    return in_maps


def kernel(x, gate_w, gate_b, params):
    from concourse.bass_utils import run_bass_kernel_spmd

    nc, meta = _get_built()
    cfg = meta["cfg"]
    T, d_in = cfg["T"], cfg["d_in"]
    in_maps = _prep_in_maps(x, gate_w, gate_b, params, cfg)

    trace = os.environ.get("BASS_MOE_TRACE", "0") == "1"
    if trace:
        _install_profile_shim()

    res = run_bass_kernel_spmd(nc, in_maps, core_ids=list(range(N_CORES)), trace=trace)
    if trace:
        _BUILT["exec_time_ns"] = res.exec_time_ns
        _BUILT["trace_path"] = (
            res.instructions_and_trace[1] if res.instructions_and_trace else None
        )

    out = np.empty((N_CORES * T, d_in), dtype=np.float32)
    for c in range(N_CORES):
        o4 = res.results[c]["outT4"].reshape(4, d_in, T)
        out[c * T : (c + 1) * T] = o4.sum(axis=0).T
    return out


def _install_profile_shim():
    """Provide antenv.axon_hooks (missing in this container) so that
    run_bass_kernel_spmd(trace=True) can capture NTFF profiles via axon."""
    import sys
    import types

    if "antenv.axon_hooks" in sys.modules:
        return
    import antenv

    mod = types.ModuleType("antenv.axon_hooks")
    mod._hook = None

    def set_axon_ntff_profile_hook(h):
        mod._hook = h

    def get_axon_ntff_profile_hook():
        return mod._hook

    mod.set_axon_ntff_profile_hook = set_axon_ntff_profile_hook
    mod.get_axon_ntff_profile_hook = get_axon_ntff_profile_hook
    sys.modules["antenv.axon_hooks"] = mod
    antenv.axon_hooks = mod

    sys.path.insert(0, "/root/.axon_site")
    from trn_agent_boot.trn_boot import _ntff_profile_via_ctypes

    hook = _ntff_profile_via_ctypes("/opt/axon/libaxon_pjrt.so")
    set_axon_ntff_profile_hook(hook)
